# revision 9
# baseline (speedup 1.0000x reference)
"""AdaptiveLIF spiking-neuron kernel for 8 TRN2 NeuronCores.

Reference recurrence (per element, over T steps):
    v = v*decay + I_t ; s = (v - vth > 0) ; v = v*(1-s)

Sharding: data-parallel over B (B=8 -> 1 batch element per core). The
recurrence is only over T, so no cross-core communication.

Per-core layout: (C,H,W) = (64,64,64) flattened to (128 partitions, 2048),
partition p holds channel c = p//2, so decay/vth are per-partition scalars.

I/O is shrunk to its floor (f32 both ways would be 32 MiB/core ~ 94 us):
  - input is read as f16 (8 MiB/core): flips 1125 of 33.5M spikes vs the
    f32 reference (rel err 1.0e-2, under the 2e-2 gate with 2x margin).
  - the output is the KEEP-mask m = (v <= vth) stored as uint8 (4 MiB);
    the host computes spikes = 1 - m.

Engine assignment (measured rates: DVE 2-operand ops 128 elem/cyc, ACT 128
elem/cyc, Pool TT ~2x slower than ACT, TensorE nearly idle):
  TensorE: v = diag(decay) @ w + Identity @ x  -> PSUM (f32 accumulate).
           f16 stationary weights; products are exact in f32. The x-matmuls
           run first (start=True) and all chunks share one ldweights; the
           w-matmuls (stop=True) each chain on the previous step's reset.
  ACT    : m = Sigmoid(-1e9*v + 1e9*vth) -> f16. The huge scale saturates
           to exactly 0.0/1.0: on this data min |1e9*(v-vth)| = 89 >> 17,
           so every mask value is exact (verified against the reference
           threshold; the 1e9*vth f32 rounding is folded into the flip
           count above).
  DVE    : w = v * m -> f16 (the reset; TT with PSUM + SBUF operands).
  Pool   : SWDGE store of m with an f16 -> uint8 cast in the DMA.
  SyncE  : input prefetch on its HWDGE ring.

The T-recurrence chain per chunk is w-matmul -> ACT -> DVE -> w-matmul;
with 4 column chunks (1 PSUM bank each, 8 banks total for double
buffering) the chain (~1.8 us) hides under the DVE throughput bound
(~2.7 us/step), giving ~16 x 2.7 = 43 us predicted.
"""

import numpy as np
from contextlib import ExitStack

import concourse.bass as bass
import concourse.tile as tile
from concourse import bacc, mybir
from concourse.bass_utils import run_bass_kernel_spmd

T, B, C, H, W = 16, 8, 64, 64, 64
P = 128                 # SBUF partitions
FD = (C * H * W) // P   # free dim per step per core = 2048
N_CORES = 8
SCALE = np.float32(1.0e9)

_nc_cache = None


def _build_nc(g=4, x_bufs=5, m_bufs=3, n_tail=2):
    ch = FD // g
    nc = bacc.Bacc("TRN2", target_bir_lowering=False, debug=False)
    f32 = mybir.dt.float32
    f16 = mybir.dt.float16
    u8 = mybir.dt.uint8
    I_ext = nc.dram_tensor("I", [T, P, FD], f16, kind="ExternalInput").ap()
    wm_ext = nc.dram_tensor("wm", [P, 256], f16, kind="ExternalInput").ap()
    bias_ext = nc.dram_tensor("bias", [P, 1], f32, kind="ExternalInput").ap()
    out_ext = nc.dram_tensor("out", [T - n_tail, P, FD], u8,
                             kind="ExternalOutput").ap()
    # Last n_tail steps go out as f16 on SyncE's HWDGE ring (it cannot cast)
    # so the expensive SWDGE dge-drain starts early and overlaps compute.
    tail_ext = nc.dram_tensor("out_tail", [n_tail, P, FD], f16,
                              kind="ExternalOutput").ap()

    with tile.TileContext(nc) as tc, ExitStack() as ctx:
        const_pool = ctx.enter_context(tc.tile_pool(name="const", bufs=1))
        state_pool = ctx.enter_context(tc.tile_pool(name="state", bufs=1))
        x_pool = ctx.enter_context(tc.tile_pool(name="x", bufs=x_bufs))
        m_pool = ctx.enter_context(tc.tile_pool(name="m", bufs=m_bufs))
        ps_pool = ctx.enter_context(tc.psum_pool(name="ps", bufs=2))

        wm = const_pool.tile([P, 256], f16, tag="wm")
        bias_sb = const_pool.tile([P, 1], f32, tag="bias")
        warm = const_pool.tile([P, 1], f16, tag="warm")
        # Load order matters: the first matmul needs wm + the first x half,
        # and each sync dma_start costs ~600 ns of issue time on SyncE.
        nc.sync.dma_start(out=wm[:], in_=wm_ext[:])
        x0a = x_pool.tile([P, FD // 2], f16, tag="x0a")
        nc.sync.dma_start(out=x0a[:], in_=I_ext[0][:, 0:FD // 2])
        nc.sync.dma_start(out=bias_sb[:], in_=bias_ext[:])
        x0b = x_pool.tile([P, FD // 2], f16, tag="x0b")
        nc.sync.dma_start(out=x0b[:], in_=I_ext[0][:, FD // 2:])
        # Pre-warm the Sigmoid ACT table (1.3 us load) under the x0 transfer.
        nc.scalar.activation(warm[:], bias_sb[:],
                             mybir.ActivationFunctionType.Sigmoid,
                             bias=0.0, scale=1.0)
        diag_w = wm[:, 0:128]
        ident_w = wm[:, 128:256]

        ws = []
        for c in range(g):
            wt = state_pool.tile([P, ch], f16, tag=f"w{c}")
            ws.append(wt)

        # Software-pipelined x-matmuls: step t issues the Identity matmuls
        # for step t+1 AFTER its own diag matmuls, so the TensorE queue is
        # [diag x4][ident x4] per step -> 2 ldweights instead of 8, and the
        # scheduler cannot interleave groups.
        def x_mms(t, xsl):
            pss = []
            for c in range(g):
                ps = ps_pool.tile([P, ch], f32, tag=f"ps{c}")
                pss.append(ps)
                nc.tensor.matmul(ps[:], ident_w, xsl(c),
                                 start=True, stop=(t == 0))
            return pss

        xh = [x0a, x0b]
        def xsl0(c):
            half = xh[(c * ch) // (FD // 2)]
            off = (c * ch) % (FD // 2)
            return half[:, off:off + ch]
        pss = x_mms(0, xsl0)

        for t in range(T):
            if t < T - 1:
                x = x_pool.tile([P, FD], f16, tag="x")
                nc.sync.dma_start(out=x[:], in_=I_ext[t + 1][:])
            m = m_pool.tile([P, FD], f16, tag="m")

            cur = pss
            for c in range(g):
                if t > 0:
                    nc.tensor.matmul(cur[c][:], diag_w, ws[c][:],
                                     start=False, stop=True)
            for c in range(g):
                # m = Sigmoid(-1e9*v + 1e9*vth): exactly 0.0/1.0 (see header)
                nc.scalar.activation(
                    m[:, c * ch:(c + 1) * ch], cur[c][:],
                    mybir.ActivationFunctionType.Sigmoid,
                    bias=bias_sb[:], scale=float(-SCALE),
                )
                if t < T - 1:
                    # reset: w = v * m  (DVE TT, PSUM x SBUF -> f16)
                    nc.vector.tensor_tensor(
                        ws[c][:], cur[c][:], m[:, c * ch:(c + 1) * ch],
                        op=mybir.AluOpType.mult,
                    )
            if t < T - 1:
                pss = x_mms(t + 1, lambda c: x[:, c * ch:(c + 1) * ch])
            if t < T - n_tail:
                # one SWDGE store per step; the DMA casts f16 -> uint8
                nc.gpsimd.dma_start(out=out_ext[t], in_=m[:])
            else:
                nc.sync.dma_start(out=tail_ext[t - (T - n_tail)], in_=m[:])

    nc.compile()
    return nc


def get_nc():
    global _nc_cache
    if _nc_cache is None:
        _nc_cache = _build_nc()
    return _nc_cache


def _prep_in_maps(I, tau, vth):
    I16 = np.asarray(I, dtype=np.float16)
    tau = np.asarray(tau, dtype=np.float32)
    vth = np.asarray(vth, dtype=np.float32)
    # Match the reference's broadcast + clamp, in fp32:
    tau_bc = np.broadcast_to(tau, (B, C)) if tau.shape[1] == 1 else tau
    vth_bc = np.broadcast_to(vth, (B, C)) if vth.shape[1] == 1 else vth
    tau_bc = np.maximum(tau_bc, np.float32(0.001))
    vth_bc = np.maximum(vth_bc, np.float32(0.001))
    decay16 = np.exp(np.float32(-1.0) / tau_bc).astype(np.float16)   # (B, C)

    in_maps = []
    rng = np.arange(P)
    for b in range(B):
        dec_p = np.repeat(decay16[b], P // C)          # (P,) f16
        vth_p = np.repeat(vth_bc[b], P // C)           # (P,) f32
        wm = np.zeros((P, 256), np.float16)
        wm[rng, rng] = dec_p
        wm[rng, 128 + rng] = np.float16(1.0)
        in_maps.append({
            "I": np.ascontiguousarray(I16[:, b]).reshape(T, P, FD),
            "wm": wm,
            "bias": (SCALE * vth_p).reshape(P, 1).astype(np.float32),
        })
    return in_maps


def run(I, tau, vth, **spmd_kwargs):
    nc = get_nc()
    in_maps = _prep_in_maps(I, tau, vth)
    res = run_bass_kernel_spmd(nc, in_maps, core_ids=list(range(N_CORES)),
                               **spmd_kwargs)
    # stored value is the keep-mask m; spikes = 1 - m
    outs = []
    for b in range(B):
        m_main = res.results[b]["out"]                       # (T-2, P, FD) u8
        m_tail = res.results[b]["out_tail"].astype(np.uint8)  # f16 0/1 -> u8
        outs.append(np.concatenate([m_main, m_tail], axis=0).reshape(T, C, H, W))
    out = np.stack(outs, axis=1)
    return (1 - out).astype(np.float32), res


def kernel(I, tau, vth):
    out, _ = run(I, tau, vth)
    return out


# revision 11
# speedup vs baseline: 1.0771x; 1.0771x over previous
"""AdaptiveLIF spiking-neuron kernel for 8 TRN2 NeuronCores.

Reference recurrence (per element, over T steps):
    v = v*decay + I_t ; s = (v - vth > 0) ; v = v*(1-s)

Sharding: data-parallel over B (B=8 -> 1 batch element per core). The
recurrence is only over T, so no cross-core communication.

Per-core layout: (C,H,W) = (64,64,64) flattened to (128 partitions, 2048),
partition p holds channel c = p//2, so decay/vth are per-partition scalars.

I/O is shrunk to its floor (f32 both ways would be 32 MiB/core ~ 94 us):
  - input is read as f16 (8 MiB/core): flips 1125 of 33.5M spikes vs the
    f32 reference (rel err 1.0e-2, under the 2e-2 gate with 2x margin).
  - the output is the KEEP-mask m = (v <= vth) stored as uint8 (4 MiB);
    the host computes spikes = 1 - m.

Engine assignment (measured rates: DVE 2-operand ops 128 elem/cyc, ACT 128
elem/cyc, Pool TT ~2x slower than ACT, TensorE nearly idle):
  TensorE: v = diag(decay) @ w + Identity @ x  -> PSUM (f32 accumulate).
           f16 stationary weights; products are exact in f32. The x-matmuls
           run first (start=True) and all chunks share one ldweights; the
           w-matmuls (stop=True) each chain on the previous step's reset.
  ACT    : m = Sigmoid(-1e9*v + 1e9*vth) -> f16. The huge scale saturates
           to exactly 0.0/1.0: on this data min |1e9*(v-vth)| = 89 >> 17,
           so every mask value is exact (verified against the reference
           threshold; the 1e9*vth f32 rounding is folded into the flip
           count above).
  DVE    : w = v * m -> f16 (the reset; TT with PSUM + SBUF operands).
  Pool   : SWDGE store of m with an f16 -> uint8 cast in the DMA.
  SyncE  : input prefetch on its HWDGE ring.

The T-recurrence chain per chunk is w-matmul -> ACT -> DVE -> w-matmul;
with 4 column chunks (1 PSUM bank each, 8 banks total for double
buffering) the chain (~1.8 us) hides under the DVE throughput bound
(~2.7 us/step), giving ~16 x 2.7 = 43 us predicted.
"""

import numpy as np
from contextlib import ExitStack

import concourse.bass as bass
import concourse.tile as tile
from concourse import bacc, mybir
from concourse.bass_utils import run_bass_kernel_spmd

T, B, C, H, W = 16, 8, 64, 64, 64
P = 128                 # SBUF partitions
FD = (C * H * W) // P   # free dim per step per core = 2048
N_CORES = 8
SCALE = np.float32(1.0e9)

_nc_cache = None


def _build_nc(g=4, x_bufs=5, m_bufs=3, n_tail=2):
    ch = FD // g
    nc = bacc.Bacc("TRN2", target_bir_lowering=False, debug=False)
    f32 = mybir.dt.float32
    f16 = mybir.dt.float16
    u8 = mybir.dt.uint8
    I_ext = nc.dram_tensor("I", [T, P, FD], f16, kind="ExternalInput").ap()
    wm_ext = nc.dram_tensor("wm", [P, 256], f16, kind="ExternalInput").ap()
    bias_ext = nc.dram_tensor("bias", [P, 1], f32, kind="ExternalInput").ap()
    out_ext = nc.dram_tensor("out", [T - n_tail, P, FD], u8,
                             kind="ExternalOutput").ap()
    # Last n_tail steps go out as f16 on SyncE's HWDGE ring (it cannot cast)
    # so the expensive SWDGE dge-drain starts early and overlaps compute.
    tail_ext = nc.dram_tensor("out_tail", [n_tail, P, FD], f16,
                              kind="ExternalOutput").ap()

    with tile.TileContext(nc) as tc, ExitStack() as ctx:
        const_pool = ctx.enter_context(tc.tile_pool(name="const", bufs=1))
        state_pool = ctx.enter_context(tc.tile_pool(name="state", bufs=1))
        x_pool = ctx.enter_context(tc.tile_pool(name="x", bufs=x_bufs))
        m_pool = ctx.enter_context(tc.tile_pool(name="m", bufs=m_bufs))
        ps_pool = ctx.enter_context(tc.psum_pool(name="ps", bufs=2))

        wm = const_pool.tile([P, 256], f16, tag="wm")
        bias_sb = const_pool.tile([P, 1], f32, tag="bias")
        warm = const_pool.tile([P, 1], f16, tag="warm")
        # Load order matters: the first matmul needs wm + the first x half,
        # and each sync dma_start costs ~600 ns of issue time on SyncE.
        nc.sync.dma_start(out=wm[:], in_=wm_ext[:])
        x0a = x_pool.tile([P, FD // 2], f16, tag="x0a")
        nc.sync.dma_start(out=x0a[:], in_=I_ext[0][:, 0:FD // 2])
        nc.sync.dma_start(out=bias_sb[:], in_=bias_ext[:])
        x0b = x_pool.tile([P, FD // 2], f16, tag="x0b")
        nc.sync.dma_start(out=x0b[:], in_=I_ext[0][:, FD // 2:])
        # Pre-warm the Sigmoid ACT table (1.3 us load) under the x0 transfer.
        nc.scalar.activation(warm[:], bias_sb[:],
                             mybir.ActivationFunctionType.Sigmoid,
                             bias=0.0, scale=1.0)
        diag_w = wm[:, 0:128]
        ident_w = wm[:, 128:256]

        ws = []
        for c in range(g):
            wt = state_pool.tile([P, ch], f16, tag=f"w{c}")
            ws.append(wt)

        # Software-pipelined x-matmuls: step t issues the Identity matmuls
        # for step t+1 AFTER its own diag matmuls, so the TensorE queue is
        # [diag x4][ident x4] per step -> 2 ldweights instead of 8, and the
        # scheduler cannot interleave groups.
        def x_mms(t, xsl):
            pss = []
            for c in range(g):
                ps = ps_pool.tile([P, ch], f32, tag=f"ps{c}")
                pss.append(ps)
                nc.tensor.matmul(ps[:], ident_w, xsl(c),
                                 start=True, stop=(t == 0))
            return pss

        xh = [x0a, x0b]
        def xsl0(c):
            half = xh[(c * ch) // (FD // 2)]
            off = (c * ch) % (FD // 2)
            return half[:, off:off + ch]
        pss = x_mms(0, xsl0)

        # x tiles keyed by step; prefetch issues 2 steps ahead so a load has
        # ~2 step-periods (~5 us) to land before its x-matmuls need it.
        xt = {}
        def prefetch(tp):
            if tp < T and tp not in xt:
                xv = x_pool.tile([P, FD], f16, tag="x")
                nc.sync.dma_start(out=xv[:], in_=I_ext[tp][:])
                xt[tp] = xv

        prefetch(1)
        for t in range(T):
            prefetch(t + 2)
            m = m_pool.tile([P, FD], f16, tag="m")

            cur = pss
            for c in range(g):
                if t > 0:
                    nc.tensor.matmul(cur[c][:], diag_w, ws[c][:],
                                     start=False, stop=True)
            for c in range(g):
                # m = Sigmoid(-1e9*v + 1e9*vth): exactly 0.0/1.0 (see header)
                nc.scalar.activation(
                    m[:, c * ch:(c + 1) * ch], cur[c][:],
                    mybir.ActivationFunctionType.Sigmoid,
                    bias=bias_sb[:], scale=float(-SCALE),
                )
                if t < T - 1:
                    # reset: w = v * m  (DVE TT, PSUM x SBUF -> f16)
                    nc.vector.tensor_tensor(
                        ws[c][:], cur[c][:], m[:, c * ch:(c + 1) * ch],
                        op=mybir.AluOpType.mult,
                    )
            if t < T - 1:
                xv = xt[t + 1]
                pss = x_mms(t + 1, lambda c: xv[:, c * ch:(c + 1) * ch])
            if t < T - n_tail:
                # one SWDGE store per step; the DMA casts f16 -> uint8
                nc.gpsimd.dma_start(out=out_ext[t], in_=m[:])
            else:
                nc.sync.dma_start(out=tail_ext[t - (T - n_tail)], in_=m[:])

    nc.compile()
    return nc


def get_nc():
    global _nc_cache
    if _nc_cache is None:
        _nc_cache = _build_nc()
    return _nc_cache


def _prep_in_maps(I, tau, vth):
    I16 = np.asarray(I, dtype=np.float16)
    tau = np.asarray(tau, dtype=np.float32)
    vth = np.asarray(vth, dtype=np.float32)
    # Match the reference's broadcast + clamp, in fp32:
    tau_bc = np.broadcast_to(tau, (B, C)) if tau.shape[1] == 1 else tau
    vth_bc = np.broadcast_to(vth, (B, C)) if vth.shape[1] == 1 else vth
    tau_bc = np.maximum(tau_bc, np.float32(0.001))
    vth_bc = np.maximum(vth_bc, np.float32(0.001))
    decay16 = np.exp(np.float32(-1.0) / tau_bc).astype(np.float16)   # (B, C)

    in_maps = []
    rng = np.arange(P)
    for b in range(B):
        dec_p = np.repeat(decay16[b], P // C)          # (P,) f16
        vth_p = np.repeat(vth_bc[b], P // C)           # (P,) f32
        wm = np.zeros((P, 256), np.float16)
        wm[rng, rng] = dec_p
        wm[rng, 128 + rng] = np.float16(1.0)
        in_maps.append({
            "I": np.ascontiguousarray(I16[:, b]).reshape(T, P, FD),
            "wm": wm,
            "bias": (SCALE * vth_p).reshape(P, 1).astype(np.float32),
        })
    return in_maps


def run(I, tau, vth, **spmd_kwargs):
    nc = get_nc()
    in_maps = _prep_in_maps(I, tau, vth)
    res = run_bass_kernel_spmd(nc, in_maps, core_ids=list(range(N_CORES)),
                               **spmd_kwargs)
    # stored value is the keep-mask m; spikes = 1 - m
    outs = []
    for b in range(B):
        m_main = res.results[b]["out"]                       # (T-2, P, FD) u8
        m_tail = res.results[b]["out_tail"].astype(np.uint8)  # f16 0/1 -> u8
        outs.append(np.concatenate([m_main, m_tail], axis=0).reshape(T, C, H, W))
    out = np.stack(outs, axis=1)
    return (1 - out).astype(np.float32), res


def kernel(I, tau, vth):
    out, _ = run(I, tau, vth)
    return out


# revision 13
# speedup vs baseline: 1.0846x; 1.0070x over previous
"""AdaptiveLIF spiking-neuron kernel for 8 TRN2 NeuronCores.

Reference recurrence (per element, over T steps):
    v = v*decay + I_t ; s = (v - vth > 0) ; v = v*(1-s)

Sharding: data-parallel over B (B=8 -> 1 batch element per core). The
recurrence is only over T, so no cross-core communication.

Per-core layout: (C,H,W) = (64,64,64) flattened to (128 partitions, 2048),
partition p holds channel c = p//2, so decay/vth are per-partition scalars.

I/O is shrunk to its floor (f32 both ways would be 32 MiB/core ~ 94 us):
  - input is read as f16 (8 MiB/core): flips 1125 of 33.5M spikes vs the
    f32 reference (rel err 1.0e-2, under the 2e-2 gate with 2x margin).
  - the output is the KEEP-mask m = (v <= vth) stored as uint8 (4 MiB);
    the host computes spikes = 1 - m.

Engine assignment (measured rates: DVE 2-operand ops 128 elem/cyc, ACT 128
elem/cyc, Pool TT ~2x slower than ACT, TensorE nearly idle):
  TensorE: v = diag(decay) @ w + Identity @ x  -> PSUM (f32 accumulate).
           f16 stationary weights; products are exact in f32. The x-matmuls
           run first (start=True) and all chunks share one ldweights; the
           w-matmuls (stop=True) each chain on the previous step's reset.
  ACT    : m = Sigmoid(-1e9*v + 1e9*vth) -> f16. The huge scale saturates
           to exactly 0.0/1.0: on this data min |1e9*(v-vth)| = 89 >> 17,
           so every mask value is exact (verified against the reference
           threshold; the 1e9*vth f32 rounding is folded into the flip
           count above).
  DVE    : w = v * m -> f16 (the reset; TT with PSUM + SBUF operands).
  Pool   : SWDGE store of m with an f16 -> uint8 cast in the DMA.
  SyncE  : input prefetch on its HWDGE ring.

The T-recurrence chain per chunk is w-matmul -> ACT -> DVE -> w-matmul;
with 4 column chunks (1 PSUM bank each, 8 banks total for double
buffering) the chain (~1.8 us) hides under the DVE throughput bound
(~2.7 us/step), giving ~16 x 2.7 = 43 us predicted.
"""

import numpy as np
from contextlib import ExitStack

import concourse.bass as bass
import concourse.tile as tile
from concourse import bacc, mybir
from concourse.bass_utils import run_bass_kernel_spmd

T, B, C, H, W = 16, 8, 64, 64, 64
P = 128                 # SBUF partitions
FD = (C * H * W) // P   # free dim per step per core = 2048
N_CORES = 8
SCALE = np.float32(1.0e9)

_nc_cache = None


def _build_nc(g=4, x_bufs=7, m_bufs=3, n_tail=2):
    ch = FD // g
    nc = bacc.Bacc("TRN2", target_bir_lowering=False, debug=False)
    f32 = mybir.dt.float32
    f16 = mybir.dt.float16
    u8 = mybir.dt.uint8
    I_ext = nc.dram_tensor("I", [T, P, FD], f16, kind="ExternalInput").ap()
    wm_ext = nc.dram_tensor("wm", [P, 256], f16, kind="ExternalInput").ap()
    bias_ext = nc.dram_tensor("bias", [P, 1], f32, kind="ExternalInput").ap()
    out_ext = nc.dram_tensor("out", [T - n_tail, P, FD], u8,
                             kind="ExternalOutput").ap()
    # Last n_tail steps go out as f16 on SyncE's HWDGE ring (it cannot cast)
    # so the expensive SWDGE dge-drain starts early and overlaps compute.
    tail_ext = nc.dram_tensor("out_tail", [n_tail, P, FD], f16,
                              kind="ExternalOutput").ap()

    with tile.TileContext(nc) as tc, ExitStack() as ctx:
        const_pool = ctx.enter_context(tc.tile_pool(name="const", bufs=1))
        state_pool = ctx.enter_context(tc.tile_pool(name="state", bufs=1))
        x_pool = ctx.enter_context(tc.tile_pool(name="x", bufs=x_bufs))
        m_pool = ctx.enter_context(tc.tile_pool(name="m", bufs=m_bufs))
        ps_pool = ctx.enter_context(tc.psum_pool(name="ps", bufs=2))

        wm = const_pool.tile([P, 256], f16, tag="wm")
        bias_sb = const_pool.tile([P, 1], f32, tag="bias")
        warm = const_pool.tile([P, 1], f16, tag="warm")
        # Load order matters: the first matmul needs wm + the first x half,
        # and each sync dma_start costs ~600 ns of issue time on SyncE.
        nc.sync.dma_start(out=wm[:], in_=wm_ext[:])
        x0a = x_pool.tile([P, FD // 2], f16, tag="x0a")
        nc.sync.dma_start(out=x0a[:], in_=I_ext[0][:, 0:FD // 2])
        nc.sync.dma_start(out=bias_sb[:], in_=bias_ext[:])
        x0b = x_pool.tile([P, FD // 2], f16, tag="x0b")
        nc.sync.dma_start(out=x0b[:], in_=I_ext[0][:, FD // 2:])
        # Pre-warm the Sigmoid ACT table (1.3 us load) under the x0 transfer.
        nc.scalar.activation(warm[:], bias_sb[:],
                             mybir.ActivationFunctionType.Sigmoid,
                             bias=0.0, scale=1.0)
        diag_w = wm[:, 0:128]
        ident_w = wm[:, 128:256]

        ws = []
        for c in range(g):
            wt = state_pool.tile([P, ch], f16, tag=f"w{c}")
            ws.append(wt)

        # Software-pipelined x-matmuls: step t issues the Identity matmuls
        # for step t+1 AFTER its own diag matmuls, so the TensorE queue is
        # [diag x4][ident x4] per step -> 2 ldweights instead of 8, and the
        # scheduler cannot interleave groups.
        def x_mms(t, xsl):
            pss = []
            for c in range(g):
                ps = ps_pool.tile([P, ch], f32, tag=f"ps{c}")
                pss.append(ps)
                nc.tensor.matmul(ps[:], ident_w, xsl(c),
                                 start=True, stop=(t == 0))
            return pss

        xh = [x0a, x0b]
        def xsl0(c):
            half = xh[(c * ch) // (FD // 2)]
            off = (c * ch) % (FD // 2)
            return half[:, off:off + ch]
        pss = x_mms(0, xsl0)

        # x tiles keyed by step; prefetch issues 2 steps ahead so a load has
        # ~2 step-periods (~5 us) to land before its x-matmuls need it.
        xt = {}
        def prefetch(tp):
            if tp < T and tp not in xt:
                xv = x_pool.tile([P, FD], f16, tag="x")
                nc.sync.dma_start(out=xv[:], in_=I_ext[tp][:])
                xt[tp] = xv

        for tp in range(1, 5):
            prefetch(tp)
        for t in range(T):
            prefetch(t + 5)
            m = m_pool.tile([P, FD], f16, tag="m")

            cur = pss
            for c in range(g):
                if t > 0:
                    nc.tensor.matmul(cur[c][:], diag_w, ws[c][:],
                                     start=False, stop=True)
            for c in range(g):
                # m = Sigmoid(-1e9*v + 1e9*vth): exactly 0.0/1.0 (see header)
                nc.scalar.activation(
                    m[:, c * ch:(c + 1) * ch], cur[c][:],
                    mybir.ActivationFunctionType.Sigmoid,
                    bias=bias_sb[:], scale=float(-SCALE),
                )
                if t < T - 1:
                    # reset: w = v * m  (DVE TT, PSUM x SBUF -> f16)
                    nc.vector.tensor_tensor(
                        ws[c][:], cur[c][:], m[:, c * ch:(c + 1) * ch],
                        op=mybir.AluOpType.mult,
                    )
            if t < T - 1:
                xv = xt[t + 1]
                pss = x_mms(t + 1, lambda c: xv[:, c * ch:(c + 1) * ch])
            if t < T - n_tail:
                # one SWDGE store per step; the DMA casts f16 -> uint8
                nc.gpsimd.dma_start(out=out_ext[t], in_=m[:])
            else:
                nc.sync.dma_start(out=tail_ext[t - (T - n_tail)], in_=m[:])

    nc.compile()
    return nc


def get_nc():
    global _nc_cache
    if _nc_cache is None:
        _nc_cache = _build_nc()
    return _nc_cache


def _prep_in_maps(I, tau, vth):
    I16 = np.asarray(I, dtype=np.float16)
    tau = np.asarray(tau, dtype=np.float32)
    vth = np.asarray(vth, dtype=np.float32)
    # Match the reference's broadcast + clamp, in fp32:
    tau_bc = np.broadcast_to(tau, (B, C)) if tau.shape[1] == 1 else tau
    vth_bc = np.broadcast_to(vth, (B, C)) if vth.shape[1] == 1 else vth
    tau_bc = np.maximum(tau_bc, np.float32(0.001))
    vth_bc = np.maximum(vth_bc, np.float32(0.001))
    decay16 = np.exp(np.float32(-1.0) / tau_bc).astype(np.float16)   # (B, C)

    in_maps = []
    rng = np.arange(P)
    for b in range(B):
        dec_p = np.repeat(decay16[b], P // C)          # (P,) f16
        vth_p = np.repeat(vth_bc[b], P // C)           # (P,) f32
        wm = np.zeros((P, 256), np.float16)
        wm[rng, rng] = dec_p
        wm[rng, 128 + rng] = np.float16(1.0)
        in_maps.append({
            "I": np.ascontiguousarray(I16[:, b]).reshape(T, P, FD),
            "wm": wm,
            "bias": (SCALE * vth_p).reshape(P, 1).astype(np.float32),
        })
    return in_maps


def run(I, tau, vth, **spmd_kwargs):
    nc = get_nc()
    in_maps = _prep_in_maps(I, tau, vth)
    res = run_bass_kernel_spmd(nc, in_maps, core_ids=list(range(N_CORES)),
                               **spmd_kwargs)
    # stored value is the keep-mask m; spikes = 1 - m
    outs = []
    for b in range(B):
        m_main = res.results[b]["out"]                       # (T-2, P, FD) u8
        m_tail = res.results[b]["out_tail"].astype(np.uint8)  # f16 0/1 -> u8
        outs.append(np.concatenate([m_main, m_tail], axis=0).reshape(T, C, H, W))
    out = np.stack(outs, axis=1)
    return (1 - out).astype(np.float32), res


def kernel(I, tau, vth):
    out, _ = run(I, tau, vth)
    return out
